# revision 15
# baseline (speedup 1.0000x reference)
"""ConvShapeletFilter kernel for Trainium2 (8 NeuronCores, data-parallel).

Math: reference computes, per batch row b and shapelet k,
    corr[b,n,k] = <x_win[b,n]-mean(x_win[b,n]), s[k]-mean(s[k])>
Since (s[k]-mean(s[k])) sums to zero over l, the window-mean term drops:
    corr[b,n,k] = sum_l x[b,n+l] * s_norm[k,l]
i.e. a plain cross-correlation with the mean-removed shapelet bank.
Outputs per (b,k): top-1, mean(top-5), top-2, relu(top1-top2) over n.

Device mapping (per core, 32 of 256 batch rows), v2 design:
  - bf16 data path (matmul accumulates fp32 in PSUM; rel-err ~1e-3,
    gate is 2e-2).
  - Full-tap hankel tile per row: H[l, f] = x[b, l + f], [128, 4160]
    bf16, one DMA per row issued on the GPSIMD engine (SWDGE) so the
    128 descriptors spray across all 16 SDMA engines by destination
    partition.  (HWDGE DIRECT2D assigned descriptors by outermost
    source-AP dim -> only 2 of 16 engines carried the im2col traffic
    in the previous version; that DMA serialization was ~95% of the
    kernel span.)
  - s_norm^T [128, 128] is the lone stationary operand (loaded once
    per matmul, never changes): corr block = snt.T @ H -> PSUM fp32.
    4 matmuls of 1024 columns per row, no accumulation splits.
  - DVE InstMax (top-8 per partition) directly on each [K, 2048] /
    [K, 1921] PSUM half-span; tiny merge InstMax.  (A fold-based
    pre-reduction was tried and reverted: random shapelets give a
    white corr profile, so top-1-per-fold-slot loses the true #2
    whenever #1/#2 sit exactly a fold distance apart — measured
    9e-2 rel err.)  All finalize ops run on ACT: p1/p2 copies,
    accumulate-mean, and dominance as Identity(p2 * -1 + bias=p1).
  - One PE transpose + 4 DMAs write y[32, 512] fp32.
"""

import os
import sys

for _p in ("/opt/trn_rl_repo", os.path.expanduser("~/.axon_site/_ro/trn_rl_repo")):
    if os.path.isdir(_p) and _p not in sys.path:
        sys.path.insert(0, _p)

import numpy as np

B, T = 256, 4096
L = 128
K = 128
K_TOP = 5
N = T - L + 1          # 3969 sliding windows
N_CORES = 8
ROWS = B // N_CORES    # 32 batch rows per core
WBLK = 512             # windows per matmul (PSUM bank = 512 fp32)
HALF = 2048            # windows per PSUM span (4 banks)
OUT_COLS = 4 * K       # p1 | p_mean | p2 | dominance
HW = 4096 + 64         # hankel tile width: f in [0, 4160)
TPAD = L + HW          # padded x row length (last read: 127 + 4159)


def _split_excess_waits(nc, mybir, max_waits=1):
    """Walrus CoreV3 codegen rejects >1 sync-wait on several instruction
    classes (CTRL/Drain, S3_LW/Matmult, ...). Hoist excess waits onto
    same-engine NoOps placed just before the offender."""
    for fn in nc.m.functions:
        for bb in fn.blocks:
            insts = bb.instructions
            i = 0
            while i < len(insts):
                inst = insts[i]
                si = inst.sync_info
                if (si is not None and si.on_wait
                        and len(si.on_wait) > max_waits):
                    waits = list(si.on_wait)
                    si.on_wait = waits[:max_waits]
                    for cs in range(max_waits, len(waits), max_waits):
                        chunk = waits[cs:cs + max_waits]
                        d = nc.sync.nop(nofuse=True)
                        cur = nc.cur_bb.bb.instructions
                        assert cur[-1] is d.ins
                        cur.pop()
                        d.ins.engine = inst.engine
                        d.ins.sync_info = mybir.SyncInfo(on_wait=chunk, on_update=[])
                        insts.insert(i, d.ins)
                        i += 1
                i += 1


def build_program():
    import concourse.bass as bass
    import concourse.mybir as mybir
    from concourse.masks import make_identity
    from concourse.tile import TileContext

    f32 = mybir.dt.float32
    bf16 = mybir.dt.bfloat16

    nc = bass.Bass()
    x = nc.declare_dram_parameter("x", [ROWS, TPAD], bf16, isOutput=False)
    snt_in = nc.declare_dram_parameter("snt", [L, K], bf16, isOutput=False)
    y = nc.declare_dram_parameter("y", [ROWS, OUT_COLS], f32, isOutput=True)

    def hankel_ap(b):
        """AP over x: dims (l, f) -> x[b, l + f]."""
        ap = x[b:b + 1, 0:HW].copy()
        ap.ap = mybir.VecI64Pair([[1, L], [1, HW]])
        ap.offset = b * TPAD
        return ap

    with TileContext(nc) as tc:
        with (
            tc.tile_pool(name="const", bufs=1) as const_pool,
            tc.tile_pool(name="hank", bufs=3) as hank_pool,
            tc.tile_pool(name="cand", bufs=3) as cand_pool,
            tc.tile_pool(name="rtop", bufs=3) as rtop_pool,
        ):
            snt = const_pool.tile([L, K], bf16)
            nc.sync.dma_start(out=snt[:, :], in_=snt_in[:, :])
            ident = const_pool.tile([128, 128], f32)
            make_identity(nc, ident[:, :])
            # Result accumulator R[k, m*32 + b], m in (p1, p_mean, p2, dom).
            R = const_pool.tile([K, 128], f32)

            # (n0, n_windows) spans; a span lives in one PSUM tile and
            # gets one InstMax.  Row 0 leads with a short span so the
            # DVE starts ~4us earlier (pipeline fill); row 31 ends with
            # a short span to shorten the drain into the output tail.
            spans_std = [(0, HALF), (HALF, N - HALF)]
            spans_first = [(0, WBLK), (WBLK, HALF - WBLK), (HALF, N - HALF)]
            spans_last = [(0, HALF), (HALF, N - HALF - WBLK),
                          (N - WBLK, WBLK)]

            with tc.tile_pool(name="psum", bufs=2, space="PSUM") as psum_pool:
                for b in range(ROWS):
                    h = hank_pool.tile([L, HW], bf16, tag="hank")
                    if b == 0:
                        # split row 0's load so the first matmul (and the
                        # DVE behind it) starts ~2.5us earlier
                        ap = hankel_ap(b)
                        ap0 = ap.copy()
                        ap0.ap = mybir.VecI64Pair([[1, L], [1, WBLK]])
                        # HWDGE: lower setup latency, and 128 outer dims
                        # still spread it across all 16 engines
                        nc.sync.dma_start(out=h[:, 0:WBLK], in_=ap0)
                        ap1 = ap.copy()
                        ap1.ap = mybir.VecI64Pair([[1, L], [1, HW - WBLK]])
                        ap1.offset = ap.offset + WBLK
                        nc.gpsimd.dma_start(out=h[:, WBLK:HW], in_=ap1)
                    else:
                        nc.gpsimd.dma_start(out=h[:, :], in_=hankel_ap(b))

                    spans = (spans_first if b == 0
                             else spans_last if b == ROWS - 1
                             else spans_std)
                    cand = cand_pool.tile([K, 8 * len(spans)], f32,
                                          tag="cand")
                    for hi, (n0, nw) in enumerate(spans):
                        ps = psum_pool.tile([K, HALF], f32, tag="psum")
                        for j in range(0, nw, WBLK):
                            w = min(WBLK, nw - j)
                            nc.tensor.matmul(
                                ps[:, j:j + w], snt[:, :],
                                h[:, n0 + j:n0 + j + w],
                                start=True, stop=True)
                        # windows >= nw are garbage (x zero-padding)
                        nc.vector.max(out=cand[:, 8 * hi:8 * (hi + 1)],
                                      in_=ps[:, :nw])

                    rt = rtop_pool.tile([K, 8], f32)
                    nc.vector.max(out=rt[:, :], in_=cand[:, :])
                    # p1, p_mean, p2, dominance -> R cols b, 32+b, 64+b, 96+b.
                    # Finalize on ACT; the DVE (bottleneck) only runs the
                    # InstMax ops above.  For the LAST row the copies +
                    # subtract go on the now-idle DVE instead, so the
                    # output tail isn't gated by a serial 4-op ACT chain.
                    last = b == ROWS - 1
                    if last:
                        nc.vector.tensor_copy(R[:, b:b + 1], rt[:, 0:1])
                        nc.vector.tensor_copy(R[:, 64 + b:65 + b], rt[:, 1:2])
                        nc.vector.tensor_sub(R[:, 96 + b:97 + b], rt[:, 0:1],
                                             rt[:, 1:2])
                    else:
                        nc.scalar.copy(R[:, b:b + 1], rt[:, 0:1])
                        nc.scalar.activation(
                            R[:, 96 + b:97 + b], rt[:, 1:2],
                            mybir.ActivationFunctionType.Identity,
                            bias=rt[:, 0:1], scale=-1.0)
                        nc.scalar.copy(R[:, 64 + b:65 + b], rt[:, 1:2])
                    pm_scratch = rtop_pool.tile([K, K_TOP], f32, tag="pmscr")
                    nc.scalar.activation(pm_scratch[:, :], rt[:, 0:K_TOP],
                                         mybir.ActivationFunctionType.Copy,
                                         scale=1.0 / K_TOP,
                                         accum_out=R[:, 32 + b:33 + b])

            # Transpose R -> TR[m*32+b, k]; write y[b, m*128+k].
            with tc.tile_pool(name="tpsum", bufs=1, space="PSUM") as tpsum_pool:
                tr_ps = tpsum_pool.tile([128, 128], f32)
                nc.tensor.transpose(tr_ps[:, :], R[:, :], ident[:, :])
                tr = const_pool.tile([128, 128], f32)
                nc.scalar.copy(tr[:, :], tr_ps[:, :])
                # split output stores across both HWDGE queues (sync +
                # scalar) to overlap their first-byte latencies
                for m in range(4):
                    eng = nc.sync if m % 2 == 0 else nc.scalar
                    eng.dma_start(out=y[:, m * K:(m + 1) * K],
                                  in_=tr[m * ROWS:(m + 1) * ROWS, :])

    _split_excess_waits(nc, mybir)
    return nc


_CACHED = {}


def _get_program():
    if "v2" not in _CACHED:
        _CACHED["v2"] = build_program()
    return _CACHED["v2"]


def _prep_inputs(x, shapelets):
    import ml_dtypes

    x = np.ascontiguousarray(x, dtype=np.float32)
    s = np.asarray(shapelets, dtype=np.float32)
    snt = np.ascontiguousarray((s - s.mean(axis=1, keepdims=True)).T)
    x = np.pad(x, ((0, 0), (0, TPAD - T)))
    return x.astype(ml_dtypes.bfloat16), snt.astype(ml_dtypes.bfloat16)


def run_sharded(x, shapelets, trace=False, **kw):
    from concourse.bass_utils import run_bass_kernel_spmd

    nc = _get_program()
    xp, snt = _prep_inputs(x, shapelets)
    in_maps = [
        {"x": xp[c * ROWS:(c + 1) * ROWS], "snt": snt}
        for c in range(N_CORES)
    ]
    res = run_bass_kernel_spmd(nc, in_maps, list(range(N_CORES)), trace=trace, **kw)
    out = np.concatenate([res.results[c]["y"] for c in range(N_CORES)], axis=0)
    return out, res


def kernel(x, shapelets):
    out, _ = run_sharded(x, shapelets)
    return out
